# revision 1
# baseline (speedup 1.0000x reference)
"""NT-Xent (SimCLR) contrastive loss on 8 Trainium2 NeuronCores.

Math: with x = row-normalized representation [8192, 256], tau = 0.5,
  sim = x @ x.T
  loss = (1/8192) * sum_i [ ln(sum_{j != i} exp(sim[i,j]/tau)) - sim[i, pos(i)]/tau ]
where pos(i) = (i + 4096) mod 8192.

Sharding: data-parallel over rows. Core c owns rows [c*1024, (c+1)*1024).
Each core receives the full representation (to build the transposed,
normalized key matrix xT in bf16), plus its own row slab and the partner
rows (i+4096 mod 8192) as separate inputs, so the single SPMD NEFF needs
no per-core control flow. Each core computes its [1024, 8192] similarity
slab with bf16 matmuls, exp+row-sum on the scalar engine (accum_out),
and the positive/diagonal terms via fp32 row-major dot products. The
per-row losses [128, 8] are DMA'd out; the host sums the 8 partials.

xT is stored as 16 column-chunk tiles of [128, 512] per K-half so phase-2
matmuls on chunk j can start as soon as its 4 source row-tiles are
transposed, overlapping with the rest of phase 1.
"""

import numpy as np
import ml_dtypes

import concourse.bacc as bacc
import concourse.bass as bass
import concourse.tile as tile
from concourse import mybir
from concourse.bass_utils import run_bass_kernel_spmd

N2 = 8192            # total rows (2N)
D = 256              # feature dim
NCORES = 8
ROWS = N2 // NCORES  # 1024 rows per core
N = N2 // 2          # positive-pair offset
P = 128              # SBUF partitions
KC = D // P          # 2 contraction chunks of 128
T_FULL = N2 // P     # 64 row tiles of the full matrix
T_SLAB = ROWS // P   # 8 row tiles of the slab
CCH = 512            # xT column-chunk width (= max matmul moving free)
NJ = N2 // CCH       # 16 column chunks
ACH = 1024           # activation chunk width (2 PSUM banks)
NJ2 = N2 // ACH      # 8 exp/rowsum chunks

F32 = mybir.dt.float32
BF16 = mybir.dt.bfloat16
AF = mybir.ActivationFunctionType
ALU = mybir.AluOpType


def _build_kernel(tc: tile.TileContext, out_ap, rep, slab, partner, ident_in):
    nc = tc.nc
    with (
        tc.tile_pool(name="const", bufs=1) as const,
        tc.tile_pool(name="persist", bufs=1) as persist,
        tc.tile_pool(name="work", bufs=4) as work,
        tc.tile_pool(name="small", bufs=4) as small,
        tc.tile_pool(name="exps", bufs=4) as exps,
        tc.tile_pool(name="pst", bufs=2, space="PSUM") as pst,
        tc.tile_pool(name="psmm", bufs=3, space="PSUM") as psmm,
    ):
        ident = const.tile([P, P], BF16, name="ident")
        nc.sync.dma_start(out=ident, in_=ident_in)
        ln2 = const.tile([P, 1], F32, name="ln2")
        nc.vector.memset(ln2, 0.6931471805599453)

        # persistent state
        xTc = [[persist.tile([P, CCH], BF16, tag=f"xT{k}_{j}", name=f"xT{k}_{j}")
                for j in range(NJ)] for k in range(KC)]
        sT = [persist.tile([P, ROWS], BF16, tag=f"sT{k}", name=f"sT{k}")
              for k in range(KC)]
        rsums = [persist.tile([P, NJ2], F32, tag=f"rs{m}", name=f"rs{m}")
                 for m in range(T_SLAB)]
        d_all = persist.tile([P, T_SLAB], F32, tag="d_all", name="d_all")
        pos2 = persist.tile([P, T_SLAB], F32, tag="pos2", name="pos2")
        sxm = persist.tile([P, T_SLAB], F32, tag="sxm", name="sxm")
        lossm = persist.tile([P, T_SLAB], F32, tag="lossm", name="lossm")

        def load_norm(src, t, out_dt, tag, logbias=None):
            """DMA row-tile t of src; return (x * exp(-0.5*ln||x||^2 + logbias),
            raw x, inv scale). logbias=ln(2) yields rows scaled by 2/||row||.
            rsqrt is computed as exp(-0.5*ln(ssq)) -- the InstReciprocal and
            tensor_scalar-with-AP-scalar paths abort on this runtime.
            """
            x = work.tile([P, D], F32, tag=f"ld_{tag}", name=f"ld_{tag}")
            nc.sync.dma_start(out=x, in_=src[t * P:(t + 1) * P, :])
            sq = work.tile([P, D], F32, tag="sq", name="sq")
            nc.vector.tensor_mul(sq, x, x)
            ssq = small.tile([P, 1], F32, tag="ssq", name="ssq")
            nc.vector.reduce_sum(ssq, sq, axis=mybir.AxisListType.X)
            lssq = small.tile([P, 1], F32, tag="lssq", name="lssq")
            nc.scalar.activation(lssq, ssq, AF.Ln)
            inv = small.tile([P, 1], F32, tag="inv", name="inv")
            nc.scalar.activation(inv, lssq, AF.Exp, scale=-0.5,
                                 bias=0.0 if logbias is None else logbias)
            xn = work.tile([P, D], out_dt, tag=f"xn_{tag}", name=f"xn_{tag}")
            nc.scalar.activation(xn, x, AF.Copy, scale=inv)
            return xn, x, inv

        def transpose_tile(xb, put):
            """xb [128, 256] bf16; put(k, psum_tile) stores the k-th half."""
            for k in range(KC):
                pt = pst.tile([P, P], BF16, tag="pt", name="pt")
                nc.tensor.transpose(pt, xb[:, k * P:(k + 1) * P], ident)
                put(k, pt)

        # phase 1a: slab rows -> sT (bf16 queries) + d_i; partner -> pos2_i
        for t in range(T_SLAB):
            xs, xraw, inv = load_norm(slab, t, F32, "slab")
            xsb = work.tile([P, D], BF16, tag="xsb", name="xsb")
            nc.scalar.activation(xsb, xraw, AF.Copy, scale=inv)
            transpose_tile(
                xsb, lambda k, pt, t=t: nc.vector.tensor_copy(
                    sT[k][:, t * P:(t + 1) * P], pt))
            sq2 = work.tile([P, D], F32, tag="sq2", name="sq2")
            nc.vector.tensor_mul(sq2, xs, xs)
            nc.vector.reduce_sum(d_all[:, t:t + 1], sq2,
                                 axis=mybir.AxisListType.X)
            xp, _, _ = load_norm(partner, t, F32, "part", logbias=ln2)
            sq3 = work.tile([P, D], F32, tag="sq3", name="sq3")
            nc.vector.tensor_mul(sq3, xs, xp)
            nc.vector.reduce_sum(pos2[:, t:t + 1], sq3,
                                 axis=mybir.AxisListType.X)

        # phase 1b: full rep -> normalized, transposed key chunks xTc (bf16)
        for t in range(T_FULL):
            xb, _, _ = load_norm(rep, t, BF16, "full")
            j, off = divmod(t * P, CCH)
            transpose_tile(
                xb, lambda k, pt, j=j, off=off: nc.vector.tensor_copy(
                    xTc[k][j][:, off:off + P], pt))

        # phase 2: similarity slab in [128, 1024] chunks; exp + row sums.
        # j2-outer so chunk j2 only needs xTc[:][2*j2:2*j2+2] (overlaps ph1b).
        for j2 in range(NJ2):
            for m in range(T_SLAB):
                ps = psmm.tile([P, ACH], F32, tag="ps", name="ps")
                for half in range(2):
                    jj = 2 * j2 + half
                    for k in range(KC):
                        nc.tensor.matmul(
                            ps[:, half * CCH:(half + 1) * CCH],
                            sT[k][:, m * P:(m + 1) * P],
                            xTc[k][jj],
                            start=(k == 0), stop=(k == KC - 1))
                esc = exps.tile([P, ACH], BF16, tag="esc", name="esc")
                nc.scalar.activation(esc, ps, AF.Exp, scale=2.0,
                                     accum_out=rsums[m][:, j2:j2 + 1])

        # tails: S_m - exp(2 d_m), then one Ln + subtract over all columns
        for m in range(T_SLAB):
            S = small.tile([P, 1], F32, tag="S", name="S")
            nc.vector.reduce_sum(S, rsums[m], axis=mybir.AxisListType.X)
            ed = small.tile([P, 1], F32, tag="ed", name="ed")
            nc.scalar.activation(ed, d_all[:, m:m + 1], AF.Exp, scale=2.0)
            nc.vector.tensor_sub(sxm[:, m:m + 1], S, ed)
        nc.scalar.activation(lossm, sxm, AF.Ln)
        nc.vector.tensor_sub(lossm, lossm, pos2)
        nc.sync.dma_start(out=out_ap, in_=lossm)


def build_nc():
    nc = bacc.Bacc("TRN2", target_bir_lowering=False, debug=False,
                   num_devices=NCORES)
    rep = nc.dram_tensor("rep", [N2, D], F32, kind="ExternalInput").ap()
    slab = nc.dram_tensor("slab", [ROWS, D], F32, kind="ExternalInput").ap()
    partner = nc.dram_tensor("partner", [ROWS, D], F32,
                             kind="ExternalInput").ap()
    ident_in = nc.dram_tensor("ident", [P, P], BF16,
                              kind="ExternalInput").ap()
    out = nc.dram_tensor("out", [P, T_SLAB], F32, kind="ExternalOutput").ap()
    with tile.TileContext(nc) as tc:
        _build_kernel(tc, out, rep, slab, partner, ident_in)
    nc.compile()
    return nc


_NC = None
LAST_RESULTS = None
_IDENT = np.eye(P, dtype=np.float32).astype(ml_dtypes.bfloat16)


def _make_in_maps(rep: np.ndarray):
    in_maps = []
    for c in range(NCORES):
        r0 = c * ROWS
        slab = np.ascontiguousarray(rep[r0:r0 + ROWS])
        pidx = (np.arange(r0, r0 + ROWS) + N) % N2
        partner = np.ascontiguousarray(rep[pidx])
        in_maps.append({"rep": rep, "slab": slab, "partner": partner,
                        "ident": _IDENT})
    return in_maps


def kernel(representation: np.ndarray, **run_kwargs) -> np.ndarray:
    global _NC, LAST_RESULTS
    rep = np.ascontiguousarray(np.asarray(representation), dtype=np.float32)
    assert rep.shape == (N2, D)
    if _NC is None:
        _NC = build_nc()
    res = run_bass_kernel_spmd(_NC, _make_in_maps(rep),
                               core_ids=list(range(NCORES)), **run_kwargs)
    LAST_RESULTS = res
    total = 0.0
    for r in res.results:
        total += float(r["out"].astype(np.float64).sum())
    return np.asarray(np.float32(total / N2))



# revision 2
# speedup vs baseline: 3.5115x; 3.5115x over previous
"""NT-Xent (SimCLR) contrastive loss on 8 Trainium2 NeuronCores.

Math: with x_hat = row-normalized representation [8192, 256], tau = 0.5,
  sim = x_hat @ x_hat.T
  loss = (1/8192) * sum_i [ ln(sum_{j!=i} exp(2 sim[i,j])) - 2 sim[i, pos(i)] ]
where pos(i) = (i + 4096) mod 8192.

Sharding: data-parallel over rows; core c owns rows [c*1024, (c+1)*1024).
The host pre-normalizes rows, quantizes to fp8e4m3 (scaled by 4), and ships
each core a pre-transposed, row-rotated key matrix xT [128, 2, 8192] so a
single SPMD program works on every core: column j of core c's slab is global
row (j + c*1024) % 8192, which puts each core's own rows at columns 0..1023
(the matmul stationary tiles) and every core's positive diagonal at columns
4096..5119.

On device, per core: 128 fp8 DoubleRow matmuls (K=256 in one pass) build the
[1024, 8192] similarity slab in [128, 2048] PSUM chunks; the ACT engine does
exp (scale folds the 1/16 fp8 scaling and the 1/tau); row-sums ride the DVE
scalar_tensor_tensor accumulator; the positive diagonal is extracted from the
u=2 chunk with an identity mask + reduce. Output is [128, 16] per core
(row-sums S and positives); the host finishes with ln(S - e^2) - 2*pos.
"""

import numpy as np
import ml_dtypes

import concourse.bacc as bacc
import concourse.bass as bass
import concourse.tile as tile
from concourse import mybir
from concourse.bass_utils import run_bass_kernel_spmd

N2 = 8192            # total rows (2N)
D = 256              # feature dim
NCORES = 8
ROWS = N2 // NCORES  # 1024 rows per core
N = N2 // 2          # positive-pair offset
P = 128              # SBUF partitions
KC = 2               # two 128-row contraction chunks (K=256 via DoubleRow)
T_SLAB = ROWS // P   # 8 row tiles of the slab
CW = 2048            # exp chunk width (4 PSUM banks)
NU = N2 // CW        # 4 chunks per slab row tile
MMW = 512            # matmul moving free width (1 PSUM bank)
FP8_SCALE = 4.0      # x_hat quantized as x_hat * 4 -> sim psum = 16*cos

F32 = mybir.dt.float32
BF16 = mybir.dt.bfloat16
FP8 = mybir.dt.float8e4
AF = mybir.ActivationFunctionType
ALU = mybir.AluOpType
DR = mybir.MatmulPerfMode.DoubleRow


def _build_kernel(tc: tile.TileContext, out_ap, xT_in, ident_in):
    nc = tc.nc
    with (
        tc.tile_pool(name="const", bufs=1) as const,
        tc.tile_pool(name="persist", bufs=1) as persist,
        tc.tile_pool(name="esc", bufs=3) as escp,
        tc.tile_pool(name="esc2", bufs=2) as esc2p,
        tc.tile_pool(name="small", bufs=2) as small,
        tc.tile_pool(name="psmm", bufs=2, space="PSUM") as psmm,
    ):
        ident = const.tile([P, P], F32, name="ident")
        nc.sync.dma_start(out=ident, in_=ident_in)

        xT = persist.tile([P, KC, N2], FP8, name="xT")
        # stream the key matrix in chunk-sized column pieces so matmuls for
        # chunk u can start as soon as its piece lands
        for u in range(NU):
            nc.sync.dma_start(out=xT[:, :, u * CW:(u + 1) * CW],
                              in_=xT_in[:, :, u * CW:(u + 1) * CW])

        rsums = persist.tile([P, T_SLAB * NU], F32, name="rsums")
        outb = persist.tile([P, 2 * T_SLAB], F32, name="outb")

        for u in range(NU):
            for m in range(T_SLAB):
                ps = psmm.tile([P, CW], F32, tag="ps", name="ps")
                for h in range(CW // MMW):
                    off = u * CW + h * MMW
                    nc.tensor.matmul(ps[:, h * MMW:(h + 1) * MMW],
                                     xT[:, :, m * P:(m + 1) * P],
                                     xT[:, :, off:off + MMW],
                                     start=True, stop=True, perf_mode=DR)
                esc = escp.tile([P, CW], BF16, tag="esc", name="esc")
                # psum holds 16*cos; exp(2*cos) = exp(psum * 0.125)
                nc.scalar.activation(esc, ps, AF.Exp, scale=2.0 / (FP8_SCALE ** 2))
                esc2 = esc2p.tile([P, CW], BF16, tag="esc2", name="esc2")
                nc.vector.scalar_tensor_tensor(
                    esc2, esc, 1.0, esc, ALU.mult, ALU.max,
                    accum_out=rsums[:, (m * NU + u):(m * NU + u) + 1])
                if u == 2:
                    # positive diagonal: cols 4096+m*128 .. +128
                    scr = small.tile([P, P], F32, tag="scr", name="scr")
                    nc.vector.tensor_mul(scr, ps[:, m * P:(m + 1) * P], ident)
                    nc.vector.reduce_sum(outb[:, T_SLAB + m:T_SLAB + m + 1],
                                         scr, axis=mybir.AxisListType.X)

        for m in range(T_SLAB):
            nc.vector.reduce_sum(outb[:, m:m + 1],
                                 rsums[:, m * NU:(m + 1) * NU],
                                 axis=mybir.AxisListType.X)
        nc.sync.dma_start(out=out_ap, in_=outb)


def build_nc():
    nc = bacc.Bacc("TRN2", target_bir_lowering=False, debug=False,
                   num_devices=NCORES)
    xT_in = nc.dram_tensor("xT", [P, KC, N2], FP8, kind="ExternalInput").ap()
    ident_in = nc.dram_tensor("ident", [P, P], F32,
                              kind="ExternalInput").ap()
    out = nc.dram_tensor("out", [P, 2 * T_SLAB], F32,
                         kind="ExternalOutput").ap()
    with tile.TileContext(nc) as tc:
        _build_kernel(tc, out, xT_in, ident_in)
    nc.compile()
    return nc


_NC = None
LAST_RESULTS = None
_IDENT = np.eye(P, dtype=np.float32)


def _make_in_maps(rep: np.ndarray):
    norm = np.maximum(np.sqrt((rep.astype(np.float64) ** 2).sum(1,
                                                                keepdims=True)),
                      1e-8)
    xh8 = (rep * (FP8_SCALE / norm)).astype(ml_dtypes.float8_e4m3)
    in_maps = []
    for c in range(NCORES):
        rot = np.roll(xh8, -c * ROWS, axis=0)  # col j = global row j + c*1024
        # xT[d, k, j] = rot[j, k*128 + d]
        xT = np.ascontiguousarray(
            rot.reshape(N2, KC, P).transpose(2, 1, 0))
        in_maps.append({"xT": xT, "ident": _IDENT})
    return in_maps


def kernel(representation: np.ndarray, **run_kwargs) -> np.ndarray:
    global _NC, LAST_RESULTS
    rep = np.ascontiguousarray(np.asarray(representation), dtype=np.float32)
    assert rep.shape == (N2, D)
    if _NC is None:
        _NC = build_nc()
    res = run_bass_kernel_spmd(_NC, _make_in_maps(rep),
                               core_ids=list(range(NCORES)), **run_kwargs)
    LAST_RESULTS = res
    total = 0.0
    e2 = float(np.exp(2.0))
    for r in res.results:
        out = r["out"].astype(np.float64)
        S = out[:, :T_SLAB]
        pos = out[:, T_SLAB:] / (FP8_SCALE ** 2)  # psum diag = 16*cos
        total += float((np.log(S - e2) - 2.0 * pos).sum())
    return np.asarray(np.float32(total / N2))


# revision 3
# speedup vs baseline: 3.6359x; 1.0354x over previous
"""NT-Xent (SimCLR) contrastive loss on 8 Trainium2 NeuronCores.

Math: with x_hat = row-normalized representation [8192, 256], tau = 0.5,
  sim = x_hat @ x_hat.T
  loss = (1/8192) * sum_i [ ln(sum_{j!=i} exp(2 sim[i,j])) - 2 sim[i, pos(i)] ]
where pos(i) = (i + 4096) mod 8192.

Sharding: data-parallel over rows; core c owns rows [c*1024, (c+1)*1024).
The host pre-normalizes rows, quantizes to fp8e4m3 (scaled by 4), and ships
each core a pre-transposed, row-rotated key matrix xT [128, 2, 8192] so a
single SPMD program works on every core: column j of core c's slab is global
row (j + c*1024) % 8192, which puts each core's own rows at columns 0..1023
(the matmul stationary tiles) and every core's positive diagonal at columns
4096..5119.

On device, per core: 128 fp8 DoubleRow matmuls (K=256 in one pass) build the
[1024, 8192] similarity slab in [128, 2048] PSUM chunks; the ACT engine does
exp (scale folds the 1/16 fp8 scaling and the 1/tau); row-sums ride the DVE
scalar_tensor_tensor accumulator; the positive diagonal is extracted from the
u=2 chunk with an identity mask + reduce. Output is [128, 16] per core
(row-sums S and positives); the host finishes with ln(S - e^2) - 2*pos.
"""

import numpy as np
import ml_dtypes

import concourse.bacc as bacc
import concourse.bass as bass
import concourse.tile as tile
from concourse import mybir
from concourse.bass_utils import run_bass_kernel_spmd

N2 = 8192            # total rows (2N)
D = 256              # feature dim
NCORES = 8
ROWS = N2 // NCORES  # 1024 rows per core
N = N2 // 2          # positive-pair offset
P = 128              # SBUF partitions
KC = 2               # two 128-row contraction chunks (K=256 via DoubleRow)
T_SLAB = ROWS // P   # 8 row tiles of the slab
CW = 2048            # exp chunk width (4 PSUM banks)
NU = N2 // CW        # 4 chunks per slab row tile
MMW = 512            # matmul moving free width (1 PSUM bank)
FP8_SCALE = 4.0      # x_hat quantized as x_hat * 4 -> sim psum = 16*cos

F32 = mybir.dt.float32
BF16 = mybir.dt.bfloat16
FP8 = mybir.dt.float8e4
AF = mybir.ActivationFunctionType
ALU = mybir.AluOpType
DR = mybir.MatmulPerfMode.DoubleRow


def _build_kernel(tc: tile.TileContext, out_ap, xT_in, ident_in):
    nc = tc.nc
    with (
        tc.tile_pool(name="const", bufs=1) as const,
        tc.tile_pool(name="persist", bufs=1) as persist,
        tc.tile_pool(name="small", bufs=2) as small,
        tc.tile_pool(name="psmm", bufs=2, space="PSUM") as psmm,
    ):
        xT = persist.tile([P, KC, N2], FP8, name="xT")
        # stream the key matrix in column pieces so matmuls can start early;
        # the first piece is small to cut the pipeline-fill latency
        splits = [0, 512, 2048, 4096, 6144, N2]
        for lo, hi in zip(splits, splits[1:]):
            nc.sync.dma_start(out=xT[:, :, lo:hi], in_=xT_in[:, :, lo:hi])
        ident = const.tile([P, P], F32, name="ident")
        nc.sync.dma_start(out=ident, in_=ident_in)

        rsums = persist.tile([P, T_SLAB * NU], F32, name="rsums")
        outb = persist.tile([P, 2 * T_SLAB], F32, name="outb")

        for u in range(NU):
            for m in range(T_SLAB):
                ps = psmm.tile([P, CW], F32, tag="ps", name="ps")
                for h in range(CW // MMW):
                    off = u * CW + h * MMW
                    nc.tensor.matmul(ps[:, h * MMW:(h + 1) * MMW],
                                     xT[:, :, m * P:(m + 1) * P],
                                     xT[:, :, off:off + MMW],
                                     start=True, stop=True, perf_mode=DR)
                if u == 2:
                    # positive diagonal: cols 4096+m*128 .. +128
                    scr = small.tile([P, P], F32, tag="scr", name="scr")
                    nc.vector.tensor_mul(scr, ps[:, m * P:(m + 1) * P], ident)
                    nc.vector.reduce_sum(outb[:, T_SLAB + m:T_SLAB + m + 1],
                                         scr, axis=mybir.AxisListType.X)
                # psum holds 16*cos; exp(2*cos) = exp(psum * 0.125), in place,
                # row-sum via the ACT accumulator
                nc.scalar.activation(ps, ps, AF.Exp,
                                     scale=2.0 / (FP8_SCALE ** 2),
                                     accum_out=rsums[:, (m * NU + u):
                                                     (m * NU + u) + 1])
                if u == NU - 1:
                    nc.vector.reduce_sum(outb[:, m:m + 1],
                                         rsums[:, m * NU:(m + 1) * NU],
                                         axis=mybir.AxisListType.X)
        nc.sync.dma_start(out=out_ap, in_=outb)


def build_nc():
    nc = bacc.Bacc("TRN2", target_bir_lowering=False, debug=False,
                   num_devices=NCORES)
    xT_in = nc.dram_tensor("xT", [P, KC, N2], FP8, kind="ExternalInput").ap()
    ident_in = nc.dram_tensor("ident", [P, P], F32,
                              kind="ExternalInput").ap()
    out = nc.dram_tensor("out", [P, 2 * T_SLAB], F32,
                         kind="ExternalOutput").ap()
    with tile.TileContext(nc) as tc:
        _build_kernel(tc, out, xT_in, ident_in)
    nc.compile()
    return nc


_NC = None
LAST_RESULTS = None
_IDENT = np.eye(P, dtype=np.float32)


def _make_in_maps(rep: np.ndarray):
    norm = np.maximum(np.sqrt((rep.astype(np.float64) ** 2).sum(1,
                                                                keepdims=True)),
                      1e-8)
    xh8 = (rep * (FP8_SCALE / norm)).astype(ml_dtypes.float8_e4m3)
    in_maps = []
    for c in range(NCORES):
        rot = np.roll(xh8, -c * ROWS, axis=0)  # col j = global row j + c*1024
        # xT[d, k, j] = rot[j, k*128 + d]
        xT = np.ascontiguousarray(
            rot.reshape(N2, KC, P).transpose(2, 1, 0))
        in_maps.append({"xT": xT, "ident": _IDENT})
    return in_maps


def kernel(representation: np.ndarray, **run_kwargs) -> np.ndarray:
    global _NC, LAST_RESULTS
    rep = np.ascontiguousarray(np.asarray(representation), dtype=np.float32)
    assert rep.shape == (N2, D)
    if _NC is None:
        _NC = build_nc()
    res = run_bass_kernel_spmd(_NC, _make_in_maps(rep),
                               core_ids=list(range(NCORES)), **run_kwargs)
    LAST_RESULTS = res
    total = 0.0
    e2 = float(np.exp(2.0))
    for r in res.results:
        out = r["out"].astype(np.float64)
        S = out[:, :T_SLAB]
        pos = out[:, T_SLAB:] / (FP8_SCALE ** 2)  # psum diag = 16*cos
        total += float((np.log(S - e2) - 2.0 * pos).sum())
    return np.asarray(np.float32(total / N2))


# revision 5
# speedup vs baseline: 3.6362x; 1.0001x over previous
"""NT-Xent (SimCLR) contrastive loss on 8 Trainium2 NeuronCores.

Math: with x_hat = row-normalized representation [8192, 256], tau = 0.5,
  sim = x_hat @ x_hat.T
  loss = (1/8192) * sum_i [ ln(sum_{j!=i} exp(2 sim[i,j])) - 2 sim[i, pos(i)] ]
where pos(i) = (i + 4096) mod 8192.

Sharding: data-parallel over rows; core c owns rows [c*1024, (c+1)*1024).
The host pre-normalizes rows, quantizes to fp8e4m3 (scaled by 4), and ships
each core a pre-transposed, row-rotated key matrix xT [128, 2, 8192] so a
single SPMD program works on every core: column j of core c's slab is global
row (j + c*1024) % 8192, which puts each core's own rows at columns 0..1023
(the matmul stationary tiles) and every core's positive diagonal at columns
4096..5119.

On device, per core: 128 fp8 DoubleRow matmuls (K=256 in one pass) build the
[1024, 8192] similarity slab in [128, 2048] PSUM chunks; the ACT engine does
exp (scale folds the 1/16 fp8 scaling and the 1/tau); row-sums ride the DVE
scalar_tensor_tensor accumulator; the positive diagonal is extracted from the
u=2 chunk with an identity mask + reduce. Output is [128, 16] per core
(row-sums S and positives); the host finishes with ln(S - e^2) - 2*pos.
"""

import numpy as np
import ml_dtypes

import concourse.bacc as bacc
import concourse.bass as bass
import concourse.tile as tile
from concourse import mybir
from concourse.bass_utils import run_bass_kernel_spmd

N2 = 8192            # total rows (2N)
D = 256              # feature dim
NCORES = 8
ROWS = N2 // NCORES  # 1024 rows per core
N = N2 // 2          # positive-pair offset
P = 128              # SBUF partitions
KC = 2               # two 128-row contraction chunks (K=256 via DoubleRow)
T_SLAB = ROWS // P   # 8 row tiles of the slab
CW = 2048            # exp chunk width (4 PSUM banks)
NU = N2 // CW        # 4 chunks per slab row tile
MMW = 512            # matmul moving free width (1 PSUM bank)
FP8_SCALE = 4.0      # x_hat quantized as x_hat * 4 -> sim psum = 16*cos

F32 = mybir.dt.float32
BF16 = mybir.dt.bfloat16
FP8 = mybir.dt.float8e4
AF = mybir.ActivationFunctionType
ALU = mybir.AluOpType
DR = mybir.MatmulPerfMode.DoubleRow


def _build_kernel(tc: tile.TileContext, out_ap, xT_in, ident_in):
    nc = tc.nc
    with (
        tc.tile_pool(name="const", bufs=1) as const,
        tc.tile_pool(name="persist", bufs=1) as persist,
        tc.tile_pool(name="small", bufs=2) as small,
        tc.tile_pool(name="esc", bufs=4) as escp,
        tc.tile_pool(name="esc2", bufs=2) as esc2p,
        tc.tile_pool(name="psmm", bufs=2, space="PSUM") as psmm,
    ):
        xT = persist.tile([P, KC, N2], FP8, name="xT")
        # stream the key matrix in column pieces so matmuls can start early;
        # the first piece is small to cut the pipeline-fill latency
        splits = [0, 512, 2048, 4096, 6144, N2]
        for lo, hi in zip(splits, splits[1:]):
            nc.sync.dma_start(out=xT[:, :, lo:hi], in_=xT_in[:, :, lo:hi])
        ident = const.tile([P, P], F32, name="ident")
        nc.sync.dma_start(out=ident, in_=ident_in)

        rsums = persist.tile([P, T_SLAB * NU], F32, name="rsums")
        outb = persist.tile([P, 2 * T_SLAB], F32, name="outb")

        for u in range(NU):
            for m in range(T_SLAB):
                ps = psmm.tile([P, CW], F32, tag="ps", name="ps")
                for h in range(CW // MMW):
                    off = u * CW + h * MMW
                    nc.tensor.matmul(ps[:, h * MMW:(h + 1) * MMW],
                                     xT[:, :, m * P:(m + 1) * P],
                                     xT[:, :, off:off + MMW],
                                     start=True, stop=True, perf_mode=DR)
                if u == 2:
                    # positive diagonal: cols 4096+m*128 .. +128
                    scr = small.tile([P, P], F32, tag="scr", name="scr")
                    nc.vector.tensor_mul(scr, ps[:, m * P:(m + 1) * P], ident)
                    nc.vector.reduce_sum(outb[:, T_SLAB + m:T_SLAB + m + 1],
                                         scr, axis=mybir.AxisListType.X)
                # psum holds 16*cos; exp(2*cos) = exp(psum * 0.125).
                # Row-sums: ACT accumulator for u==0 chunks, DVE STT-accum for
                # the rest — splits the accumulation cost across both engines.
                ri = rsums[:, (m * NU + u):(m * NU + u) + 1]
                esc = escp.tile([P, CW], BF16, tag="esc", name="esc")
                if u == 0:
                    nc.scalar.activation(esc, ps, AF.Exp,
                                         scale=2.0 / (FP8_SCALE ** 2),
                                         accum_out=ri)
                else:
                    nc.scalar.activation(esc, ps, AF.Exp,
                                         scale=2.0 / (FP8_SCALE ** 2))
                    esc2 = esc2p.tile([P, CW], BF16, tag="esc2", name="esc2")
                    nc.vector.scalar_tensor_tensor(
                        esc2, esc, 1.0, esc, ALU.mult, ALU.max, accum_out=ri)
                if u == NU - 1:
                    nc.vector.reduce_sum(outb[:, m:m + 1],
                                         rsums[:, m * NU:(m + 1) * NU],
                                         axis=mybir.AxisListType.X)
        nc.sync.dma_start(out=out_ap, in_=outb)


def build_nc():
    nc = bacc.Bacc("TRN2", target_bir_lowering=False, debug=False,
                   num_devices=NCORES)
    xT_in = nc.dram_tensor("xT", [P, KC, N2], FP8, kind="ExternalInput").ap()
    ident_in = nc.dram_tensor("ident", [P, P], F32,
                              kind="ExternalInput").ap()
    out = nc.dram_tensor("out", [P, 2 * T_SLAB], F32,
                         kind="ExternalOutput").ap()
    with tile.TileContext(nc) as tc:
        _build_kernel(tc, out, xT_in, ident_in)
    nc.compile()
    return nc


_NC = None
LAST_RESULTS = None
_IDENT = np.eye(P, dtype=np.float32)


def _make_in_maps(rep: np.ndarray):
    norm = np.maximum(np.sqrt((rep.astype(np.float64) ** 2).sum(1,
                                                                keepdims=True)),
                      1e-8)
    xh8 = (rep * (FP8_SCALE / norm)).astype(ml_dtypes.float8_e4m3)
    in_maps = []
    for c in range(NCORES):
        rot = np.roll(xh8, -c * ROWS, axis=0)  # col j = global row j + c*1024
        # xT[d, k, j] = rot[j, k*128 + d]
        xT = np.ascontiguousarray(
            rot.reshape(N2, KC, P).transpose(2, 1, 0))
        in_maps.append({"xT": xT, "ident": _IDENT})
    return in_maps


def kernel(representation: np.ndarray, **run_kwargs) -> np.ndarray:
    global _NC, LAST_RESULTS
    rep = np.ascontiguousarray(np.asarray(representation), dtype=np.float32)
    assert rep.shape == (N2, D)
    if _NC is None:
        _NC = build_nc()
    res = run_bass_kernel_spmd(_NC, _make_in_maps(rep),
                               core_ids=list(range(NCORES)), **run_kwargs)
    LAST_RESULTS = res
    total = 0.0
    e2 = float(np.exp(2.0))
    for r in res.results:
        out = r["out"].astype(np.float64)
        S = out[:, :T_SLAB]
        pos = out[:, T_SLAB:] / (FP8_SCALE ** 2)  # psum diag = 16*cos
        total += float((np.log(S - e2) - 2.0 * pos).sum())
    return np.asarray(np.float32(total / N2))


# revision 6
# speedup vs baseline: 3.7549x; 1.0327x over previous
"""NT-Xent (SimCLR) contrastive loss on 8 Trainium2 NeuronCores.

Math: with x_hat = row-normalized representation [8192, 256], tau = 0.5,
  sim = x_hat @ x_hat.T
  loss = (1/8192) * sum_i [ ln(sum_{j!=i} exp(2 sim[i,j])) - 2 sim[i, pos(i)] ]
where pos(i) = (i + 4096) mod 8192.

Sharding: data-parallel over rows; core c owns rows [c*1024, (c+1)*1024).
The host pre-normalizes rows, quantizes to fp8e4m3 (scaled by 4), and ships
each core a pre-transposed, row-rotated key matrix xT [128, 2, 8192] so a
single SPMD program works on every core: column j of core c's slab is global
row (j + c*1024) % 8192, which puts each core's own rows at columns 0..1023
(the matmul stationary tiles) and every core's positive diagonal at columns
4096..5119.

On device, per core: 128 fp8 DoubleRow matmuls (K=256 in one pass) build the
[1024, 8192] similarity slab in [128, 2048] PSUM chunks; the ACT engine does
exp (scale folds the 1/16 fp8 scaling and the 1/tau); row-sums ride the DVE
scalar_tensor_tensor accumulator; the positive diagonal is extracted from the
u=2 chunk with an identity mask + reduce. Output is [128, 16] per core
(row-sums S and positives); the host finishes with ln(S - e^2) - 2*pos.
"""

import numpy as np
import ml_dtypes

import concourse.bacc as bacc
import concourse.bass as bass
import concourse.tile as tile
from concourse import mybir
from concourse.bass_utils import run_bass_kernel_spmd

N2 = 8192            # total rows (2N)
D = 256              # feature dim
NCORES = 8
ROWS = N2 // NCORES  # 1024 rows per core
N = N2 // 2          # positive-pair offset
P = 128              # SBUF partitions
KC = 2               # two 128-row contraction chunks (K=256 via DoubleRow)
T_SLAB = ROWS // P   # 8 row tiles of the slab
CW = 2048            # exp chunk width (4 PSUM banks)
NU = N2 // CW        # 4 chunks per slab row tile
MMW = 512            # matmul moving free width (1 PSUM bank)
FP8_SCALE = 4.0      # x_hat quantized as x_hat * 4 -> sim psum = 16*cos

F32 = mybir.dt.float32
BF16 = mybir.dt.bfloat16
FP8 = mybir.dt.float8e4
AF = mybir.ActivationFunctionType
ALU = mybir.AluOpType
DR = mybir.MatmulPerfMode.DoubleRow


def _build_kernel(tc: tile.TileContext, out_ap, xT_in, ident_in):
    nc = tc.nc
    with (
        tc.tile_pool(name="const", bufs=1) as const,
        tc.tile_pool(name="persist", bufs=1) as persist,
        tc.tile_pool(name="small", bufs=2) as small,
        tc.tile_pool(name="esc", bufs=8) as escp,
        tc.tile_pool(name="esc2", bufs=4) as esc2p,
        tc.tile_pool(name="psmm", bufs=2, space="PSUM") as psmm,
    ):
        xT = persist.tile([P, KC, N2], FP8, name="xT")
        # stream the key matrix in column pieces so matmuls can start early;
        # the first piece is small to cut the pipeline-fill latency
        splits = [0, 512, 2048, 4096, 6144, N2]
        for lo, hi in zip(splits, splits[1:]):
            nc.sync.dma_start(out=xT[:, :, lo:hi], in_=xT_in[:, :, lo:hi])
        ident = const.tile([P, P], F32, name="ident")
        nc.sync.dma_start(out=ident, in_=ident_in)

        rsums = persist.tile([P, T_SLAB * NU], F32, name="rsums")
        outb = persist.tile([P, 2 * T_SLAB], F32, name="outb")

        for u in range(NU):
            for m in range(T_SLAB):
                ps = psmm.tile([P, CW], F32, tag="ps", name="ps")
                for h in range(CW // MMW):
                    off = u * CW + h * MMW
                    nc.tensor.matmul(ps[:, h * MMW:(h + 1) * MMW],
                                     xT[:, :, m * P:(m + 1) * P],
                                     xT[:, :, off:off + MMW],
                                     start=True, stop=True, perf_mode=DR)
                if u == 2:
                    # positive diagonal: cols 4096+m*128 .. +128
                    scr = small.tile([P, P], F32, tag="scr", name="scr")
                    nc.vector.tensor_mul(scr, ps[:, m * P:(m + 1) * P], ident)
                    nc.vector.reduce_sum(outb[:, T_SLAB + m:T_SLAB + m + 1],
                                         scr, axis=mybir.AxisListType.X)
                # psum holds 16*cos; exp(2*cos) = exp(psum * 0.125).
                # Row-sums: ACT accumulator for u==0 chunks, DVE STT-accum for
                # the rest — splits the accumulation cost across both engines.
                ri = rsums[:, (m * NU + u):(m * NU + u) + 1]
                esc = escp.tile([P, CW], BF16, tag="esc", name="esc")
                if u in (0, NU - 1):
                    nc.scalar.activation(esc, ps, AF.Exp,
                                         scale=2.0 / (FP8_SCALE ** 2),
                                         accum_out=ri)
                else:
                    nc.scalar.activation(esc, ps, AF.Exp,
                                         scale=2.0 / (FP8_SCALE ** 2))
                    esc2 = esc2p.tile([P, CW], BF16, tag="esc2", name="esc2")
                    nc.vector.scalar_tensor_tensor(
                        esc2, esc, 1.0, esc, ALU.mult, ALU.max, accum_out=ri)
                if u == NU - 1:
                    nc.vector.reduce_sum(outb[:, m:m + 1],
                                         rsums[:, m * NU:(m + 1) * NU],
                                         axis=mybir.AxisListType.X)
        nc.sync.dma_start(out=out_ap, in_=outb)


def build_nc():
    nc = bacc.Bacc("TRN2", target_bir_lowering=False, debug=False,
                   num_devices=NCORES)
    xT_in = nc.dram_tensor("xT", [P, KC, N2], FP8, kind="ExternalInput").ap()
    ident_in = nc.dram_tensor("ident", [P, P], F32,
                              kind="ExternalInput").ap()
    out = nc.dram_tensor("out", [P, 2 * T_SLAB], F32,
                         kind="ExternalOutput").ap()
    with tile.TileContext(nc) as tc:
        _build_kernel(tc, out, xT_in, ident_in)
    nc.compile()
    return nc


_NC = None
LAST_RESULTS = None
_IDENT = np.eye(P, dtype=np.float32)


def _make_in_maps(rep: np.ndarray):
    norm = np.maximum(np.sqrt((rep.astype(np.float64) ** 2).sum(1,
                                                                keepdims=True)),
                      1e-8)
    xh8 = (rep * (FP8_SCALE / norm)).astype(ml_dtypes.float8_e4m3)
    in_maps = []
    for c in range(NCORES):
        rot = np.roll(xh8, -c * ROWS, axis=0)  # col j = global row j + c*1024
        # xT[d, k, j] = rot[j, k*128 + d]
        xT = np.ascontiguousarray(
            rot.reshape(N2, KC, P).transpose(2, 1, 0))
        in_maps.append({"xT": xT, "ident": _IDENT})
    return in_maps


def kernel(representation: np.ndarray, **run_kwargs) -> np.ndarray:
    global _NC, LAST_RESULTS
    rep = np.ascontiguousarray(np.asarray(representation), dtype=np.float32)
    assert rep.shape == (N2, D)
    if _NC is None:
        _NC = build_nc()
    res = run_bass_kernel_spmd(_NC, _make_in_maps(rep),
                               core_ids=list(range(NCORES)), **run_kwargs)
    LAST_RESULTS = res
    total = 0.0
    e2 = float(np.exp(2.0))
    for r in res.results:
        out = r["out"].astype(np.float64)
        S = out[:, :T_SLAB]
        pos = out[:, T_SLAB:] / (FP8_SCALE ** 2)  # psum diag = 16*cos
        total += float((np.log(S - e2) - 2.0 * pos).sum())
    return np.asarray(np.float32(total / N2))


# revision 7
# speedup vs baseline: 4.5157x; 1.2026x over previous
"""NT-Xent (SimCLR) contrastive loss on 8 Trainium2 NeuronCores.

Math: with x_hat = row-normalized representation [8192, 256], tau = 0.5,
  sim = x_hat @ x_hat.T
  loss = (1/8192) * sum_i [ ln(sum_{j!=i} exp(2 sim[i,j])) - 2 sim[i, pos(i)] ]
where pos(i) = (i + 4096) mod 8192.

Sharding: data-parallel over rows; core c owns rows [c*1024, (c+1)*1024).
The host pre-normalizes rows, quantizes to fp8e4m3 (scaled by 4), and ships
each core a pre-transposed, row-rotated key matrix xT [128, 2, 8192] so a
single SPMD program works on every core: column j of core c's slab is global
row (j + c*1024) % 8192, which puts each core's own rows at columns 0..1023
(the matmul stationary tiles) and every core's positive diagonal at columns
4096..5119.

On device, per core: 128 fp8 DoubleRow matmuls (K=256 in one pass) build the
[1024, 8192] similarity slab in [128, 2048] PSUM chunks; the ACT engine does
exp (the scale folds the 1/16 fp8 scaling and 1/tau); row-sums ride the DVE
scalar_tensor_tensor accumulator except the last few chunks, which use the
ACT accumulator so the kernel doesn't end DVE-bound. The positive diagonal is
read lazily from the exp'd u=2 chunk (identity mask + reduce; the host takes
ln to recover 2*cos). Output is [128, 16] per core; the host finishes with
ln(S - e^2) - ln(pos_exp) summed over rows.
"""

import numpy as np
import ml_dtypes

import concourse.bacc as bacc
import concourse.bass as bass
import concourse.tile as tile
from concourse import mybir
from concourse.bass_utils import run_bass_kernel_spmd

N2 = 8192            # total rows (2N)
D = 256              # feature dim
NCORES = 8
ROWS = N2 // NCORES  # 1024 rows per core
N = N2 // 2          # positive-pair offset
P = 128              # SBUF partitions
KC = 2               # two 128-row contraction chunks (K=256 via DoubleRow)
T_SLAB = ROWS // P   # 8 row tiles of the slab
CW = 2048            # exp chunk width (4 PSUM banks)
NU = N2 // CW        # 4 chunks per slab row tile
MMW = 512            # matmul moving free width (1 PSUM bank)
FP8_SCALE = 4.0      # x_hat quantized as x_hat * 4 -> sim psum = 16*cos

F32 = mybir.dt.float32
BF16 = mybir.dt.bfloat16
FP8 = mybir.dt.float8e4
AF = mybir.ActivationFunctionType
ALU = mybir.AluOpType
DR = mybir.MatmulPerfMode.DoubleRow


def _build_kernel(tc: tile.TileContext, out_ap, xT_in, ident_in):
    nc = tc.nc
    with (
        tc.tile_pool(name="sb", bufs=1) as sb,
        tc.tile_pool(name="psmm", bufs=2, space="PSUM") as psmm,
    ):
        xT = sb.tile([P, KC, N2], FP8, name="xT")
        # stream the key matrix in column pieces so matmuls can start early;
        # the first piece is small to cut the pipeline-fill latency
        splits = [0, 512, 2048, 4096, 6144, N2]
        for lo, hi in zip(splits, splits[1:]):
            nc.sync.dma_start(out=xT[:, :, lo:hi], in_=xT_in[:, :, lo:hi])
        ident = sb.tile([P, P], BF16, name="ident")
        nc.sync.dma_start(out=ident, in_=ident_in)

        rsums = sb.tile([P, T_SLAB * NU], F32, name="rsums")
        outb = sb.tile([P, 2 * T_SLAB], F32, name="outb")

        for u in range(NU):
            for m in range(T_SLAB):
                ps = psmm.tile([P, CW], F32, tag="ps", name="ps")
                for h in range(CW // MMW):
                    off = u * CW + h * MMW
                    nc.tensor.matmul(ps[:, h * MMW:(h + 1) * MMW],
                                     xT[:, :, m * P:(m + 1) * P],
                                     xT[:, :, off:off + MMW],
                                     start=True, stop=True, perf_mode=DR)
                # psum holds 16*cos; exp(2*cos) = exp(psum * 0.125).
                # Row-sums ride the DVE STT accumulator, except the last
                # T_SLAB-1 chunks which use the ACT accumulator (the DVE
                # backlog would otherwise trail the final exp).
                ri = rsums[:, (m * NU + u):(m * NU + u) + 1]
                esc = sb.tile([P, CW], BF16, tag="esc", name="esc", bufs=12)
                if u == NU - 1 and m >= 1:
                    nc.scalar.activation(esc, ps, AF.Exp,
                                         scale=2.0 / (FP8_SCALE ** 2),
                                         accum_out=ri)
                else:
                    nc.scalar.activation(esc, ps, AF.Exp,
                                         scale=2.0 / (FP8_SCALE ** 2))
                    esc2 = sb.tile([P, CW], BF16, tag="esc2", name="esc2",
                                   bufs=4)
                    nc.vector.scalar_tensor_tensor(
                        esc2, esc, 1.0, esc, ALU.mult, ALU.max, accum_out=ri)
                if u == 2:
                    # positive diagonal (cols 4096+m*128..+128) from the exp'd
                    # chunk: host recovers 2*cos with a log
                    scr = sb.tile([P, P], BF16, tag="scr", name="scr", bufs=2)
                    nc.vector.tensor_mul(scr, esc[:, m * P:(m + 1) * P],
                                         ident)
                    nc.vector.reduce_sum(outb[:, T_SLAB + m:T_SLAB + m + 1],
                                         scr, axis=mybir.AxisListType.X)
                if u == NU - 1:
                    nc.vector.reduce_sum(outb[:, m:m + 1],
                                         rsums[:, m * NU:(m + 1) * NU],
                                         axis=mybir.AxisListType.X)
        nc.sync.dma_start(out=out_ap, in_=outb)


def build_nc():
    nc = bacc.Bacc("TRN2", target_bir_lowering=False, debug=False,
                   num_devices=NCORES)
    xT_in = nc.dram_tensor("xT", [P, KC, N2], FP8, kind="ExternalInput").ap()
    ident_in = nc.dram_tensor("ident", [P, P], BF16,
                              kind="ExternalInput").ap()
    out = nc.dram_tensor("out", [P, 2 * T_SLAB], F32,
                         kind="ExternalOutput").ap()
    with tile.TileContext(nc) as tc:
        _build_kernel(tc, out, xT_in, ident_in)
    nc.compile()
    return nc


_NC = None
LAST_RESULTS = None
_IDENT = np.eye(P, dtype=np.float32).astype(ml_dtypes.bfloat16)


def _make_in_maps(rep: np.ndarray):
    norm = np.maximum(np.sqrt((rep.astype(np.float64) ** 2).sum(1,
                                                                keepdims=True)),
                      1e-8)
    xh8 = (rep * (FP8_SCALE / norm)).astype(ml_dtypes.float8_e4m3)
    in_maps = []
    for c in range(NCORES):
        rot = np.roll(xh8, -c * ROWS, axis=0)  # col j = global row j + c*1024
        # xT[d, k, j] = rot[j, k*128 + d]
        xT = np.ascontiguousarray(
            rot.reshape(N2, KC, P).transpose(2, 1, 0))
        in_maps.append({"xT": xT, "ident": _IDENT})
    return in_maps


def kernel(representation: np.ndarray, **run_kwargs) -> np.ndarray:
    global _NC, LAST_RESULTS
    rep = np.ascontiguousarray(np.asarray(representation), dtype=np.float32)
    assert rep.shape == (N2, D)
    if _NC is None:
        _NC = build_nc()
    res = run_bass_kernel_spmd(_NC, _make_in_maps(rep),
                               core_ids=list(range(NCORES)), **run_kwargs)
    LAST_RESULTS = res
    total = 0.0
    e2 = float(np.exp(2.0))
    for r in res.results:
        out = r["out"].astype(np.float64)
        S = out[:, :T_SLAB]
        pos_exp = out[:, T_SLAB:]          # = exp(2*cos) of the positive pair
        total += float((np.log(S - e2) - np.log(pos_exp)).sum())
    return np.asarray(np.float32(total / N2))


# revision 8
# speedup vs baseline: 4.5536x; 1.0084x over previous
"""NT-Xent (SimCLR) contrastive loss on 8 Trainium2 NeuronCores.

Math: with x_hat = row-normalized representation [8192, 256], tau = 0.5,
  sim = x_hat @ x_hat.T
  loss = (1/8192) * sum_i [ ln(sum_{j!=i} exp(2 sim[i,j])) - 2 sim[i, pos(i)] ]
where pos(i) = (i + 4096) mod 8192.

Sharding: data-parallel over rows; core c owns rows [c*1024, (c+1)*1024).
The host pre-normalizes rows, quantizes to fp8e4m3 (scaled by 4), and ships
each core a pre-transposed, row-rotated key matrix xT [128, 2, 8192] so a
single SPMD program works on every core: column j of core c's slab is global
row (j + c*1024) % 8192, which puts each core's own rows at columns 0..1023
(the matmul stationary tiles) and every core's positive diagonal at columns
4096..5119.

On device, per core: 128 fp8 DoubleRow matmuls (K=256 in one pass) build the
[1024, 8192] similarity slab in [128, 2048] PSUM chunks; the ACT engine does
exp (the scale folds the 1/16 fp8 scaling and 1/tau); row-sums ride the DVE
scalar_tensor_tensor accumulator except the last few chunks, which use the
ACT accumulator so the kernel doesn't end DVE-bound. The positive diagonal is
read lazily from the exp'd u=2 chunk (identity mask + reduce; the host takes
ln to recover 2*cos). Output is [128, 16] per core; the host finishes with
ln(S - e^2) - ln(pos_exp) summed over rows.
"""

import numpy as np
import ml_dtypes

import concourse.bacc as bacc
import concourse.bass as bass
import concourse.tile as tile
from concourse import mybir
from concourse.bass_utils import run_bass_kernel_spmd

N2 = 8192            # total rows (2N)
D = 256              # feature dim
NCORES = 8
ROWS = N2 // NCORES  # 1024 rows per core
N = N2 // 2          # positive-pair offset
P = 128              # SBUF partitions
KC = 2               # two 128-row contraction chunks (K=256 via DoubleRow)
T_SLAB = ROWS // P   # 8 row tiles of the slab
CW = 2048            # exp chunk width (4 PSUM banks)
NU = N2 // CW        # 4 chunks per slab row tile
MMW = 512            # matmul moving free width (1 PSUM bank)
FP8_SCALE = 4.0      # x_hat quantized as x_hat * 4 -> sim psum = 16*cos

F32 = mybir.dt.float32
BF16 = mybir.dt.bfloat16
FP8 = mybir.dt.float8e4
AF = mybir.ActivationFunctionType
ALU = mybir.AluOpType
DR = mybir.MatmulPerfMode.DoubleRow


def _build_kernel(tc: tile.TileContext, out_ap, xT_in, ident_in):
    nc = tc.nc
    with (
        tc.tile_pool(name="sb", bufs=1) as sb,
        tc.tile_pool(name="psmm", bufs=2, space="PSUM") as psmm,
    ):
        xT = sb.tile([P, KC, N2], FP8, name="xT")
        # stream the key matrix in column pieces so matmuls can start early;
        # the first piece is small to cut the pipeline-fill latency
        splits = [0, 256, 512, 1024, 2048, 3072, 4096, 5120, 6144, 7168, N2]
        for lo, hi in zip(splits, splits[1:]):
            nc.sync.dma_start(out=xT[:, :, lo:hi], in_=xT_in[:, :, lo:hi])
        ident = sb.tile([P, P], BF16, name="ident")
        nc.sync.dma_start(out=ident, in_=ident_in)

        rsums = sb.tile([P, T_SLAB * NU], F32, name="rsums")
        outb = sb.tile([P, 2 * T_SLAB], F32, name="outb")

        for u in range(NU):
            for m in range(T_SLAB):
                ps = psmm.tile([P, CW], F32, tag="ps", name="ps")
                for h in range(CW // MMW):
                    off = u * CW + h * MMW
                    nc.tensor.matmul(ps[:, h * MMW:(h + 1) * MMW],
                                     xT[:, :, m * P:(m + 1) * P],
                                     xT[:, :, off:off + MMW],
                                     start=True, stop=True, perf_mode=DR)
                # psum holds 16*cos; exp(2*cos) = exp(psum * 0.125).
                # Row-sums ride the DVE STT accumulator, except the last
                # T_SLAB-1 chunks which use the ACT accumulator (the DVE
                # backlog would otherwise trail the final exp).
                ri = rsums[:, (m * NU + u):(m * NU + u) + 1]
                esc = sb.tile([P, CW], BF16, tag="esc", name="esc", bufs=12)
                if u == NU - 1 and m >= 1:
                    nc.scalar.activation(esc, ps, AF.Exp,
                                         scale=2.0 / (FP8_SCALE ** 2),
                                         accum_out=ri)
                else:
                    nc.scalar.activation(esc, ps, AF.Exp,
                                         scale=2.0 / (FP8_SCALE ** 2))
                    esc2 = sb.tile([P, CW], BF16, tag="esc2", name="esc2",
                                   bufs=4)
                    nc.vector.scalar_tensor_tensor(
                        esc2, esc, 1.0, esc, ALU.mult, ALU.max, accum_out=ri)
                if u == 2:
                    # positive diagonal (cols 4096+m*128..+128) from the exp'd
                    # chunk: host recovers 2*cos with a log
                    scr = sb.tile([P, P], BF16, tag="scr", name="scr", bufs=2)
                    nc.vector.tensor_mul(scr, esc[:, m * P:(m + 1) * P],
                                         ident)
                    nc.vector.reduce_sum(outb[:, T_SLAB + m:T_SLAB + m + 1],
                                         scr, axis=mybir.AxisListType.X)
                if u == NU - 1:
                    nc.vector.reduce_sum(outb[:, m:m + 1],
                                         rsums[:, m * NU:(m + 1) * NU],
                                         axis=mybir.AxisListType.X)
        nc.sync.dma_start(out=out_ap, in_=outb)


def build_nc():
    nc = bacc.Bacc("TRN2", target_bir_lowering=False, debug=False,
                   num_devices=NCORES)
    xT_in = nc.dram_tensor("xT", [P, KC, N2], FP8, kind="ExternalInput").ap()
    ident_in = nc.dram_tensor("ident", [P, P], BF16,
                              kind="ExternalInput").ap()
    out = nc.dram_tensor("out", [P, 2 * T_SLAB], F32,
                         kind="ExternalOutput").ap()
    with tile.TileContext(nc) as tc:
        _build_kernel(tc, out, xT_in, ident_in)
    nc.compile()
    return nc


_NC = None
LAST_RESULTS = None
_IDENT = np.eye(P, dtype=np.float32).astype(ml_dtypes.bfloat16)


def _make_in_maps(rep: np.ndarray):
    norm = np.maximum(np.sqrt((rep.astype(np.float64) ** 2).sum(1,
                                                                keepdims=True)),
                      1e-8)
    xh8 = (rep * (FP8_SCALE / norm)).astype(ml_dtypes.float8_e4m3)
    in_maps = []
    for c in range(NCORES):
        rot = np.roll(xh8, -c * ROWS, axis=0)  # col j = global row j + c*1024
        # xT[d, k, j] = rot[j, k*128 + d]
        xT = np.ascontiguousarray(
            rot.reshape(N2, KC, P).transpose(2, 1, 0))
        in_maps.append({"xT": xT, "ident": _IDENT})
    return in_maps


def kernel(representation: np.ndarray, **run_kwargs) -> np.ndarray:
    global _NC, LAST_RESULTS
    rep = np.ascontiguousarray(np.asarray(representation), dtype=np.float32)
    assert rep.shape == (N2, D)
    if _NC is None:
        _NC = build_nc()
    res = run_bass_kernel_spmd(_NC, _make_in_maps(rep),
                               core_ids=list(range(NCORES)), **run_kwargs)
    LAST_RESULTS = res
    total = 0.0
    e2 = float(np.exp(2.0))
    for r in res.results:
        out = r["out"].astype(np.float64)
        S = out[:, :T_SLAB]
        pos_exp = out[:, T_SLAB:]          # = exp(2*cos) of the positive pair
        total += float((np.log(S - e2) - np.log(pos_exp)).sum())
    return np.asarray(np.float32(total / N2))


# revision 9
# speedup vs baseline: 4.5938x; 1.0088x over previous
"""NT-Xent (SimCLR) contrastive loss on 8 Trainium2 NeuronCores.

Math: with x_hat = row-normalized representation [8192, 256], tau = 0.5,
  sim = x_hat @ x_hat.T
  loss = (1/8192) * sum_i [ ln(sum_{j!=i} exp(2 sim[i,j])) - 2 sim[i, pos(i)] ]
where pos(i) = (i + 4096) mod 8192.

Sharding: data-parallel over rows; core c owns rows [c*1024, (c+1)*1024).
The host pre-normalizes rows, quantizes to fp8e4m3 (scaled by 4), and ships
each core a pre-transposed, row-rotated key matrix xT [128, 2, 8192] so a
single SPMD program works on every core: column j of core c's slab is global
row (j + c*1024) % 8192, which puts each core's own rows at columns 0..1023
(the matmul stationary tiles) and every core's positive diagonal at columns
4096..5119.

On device, per core: 128 fp8 DoubleRow matmuls (K=256 in one pass) build the
[1024, 8192] similarity slab in [128, 2048] PSUM chunks; the ACT engine does
exp (the scale folds the 1/16 fp8 scaling and 1/tau); row-sums ride the DVE
scalar_tensor_tensor accumulator except the last few chunks, which use the
ACT accumulator so the kernel doesn't end DVE-bound. The positive diagonal is
read lazily from the exp'd u=2 chunk (identity mask + reduce; the host takes
ln to recover 2*cos). Output is [128, 16] per core; the host finishes with
ln(S - e^2) - ln(pos_exp) summed over rows.
"""

import numpy as np
import ml_dtypes

import concourse.bacc as bacc
import concourse.bass as bass
import concourse.tile as tile
from concourse import mybir
from concourse.bass_utils import run_bass_kernel_spmd

N2 = 8192            # total rows (2N)
D = 256              # feature dim
NCORES = 8
ROWS = N2 // NCORES  # 1024 rows per core
N = N2 // 2          # positive-pair offset
P = 128              # SBUF partitions
KC = 2               # two 128-row contraction chunks (K=256 via DoubleRow)
T_SLAB = ROWS // P   # 8 row tiles of the slab
CW = 2048            # exp chunk width (4 PSUM banks)
NU = N2 // CW        # 4 chunks per slab row tile
MMW = 512            # matmul moving free width (1 PSUM bank)
FP8_SCALE = 4.0      # x_hat quantized as x_hat * 4 -> sim psum = 16*cos

F32 = mybir.dt.float32
BF16 = mybir.dt.bfloat16
FP8 = mybir.dt.float8e4
AF = mybir.ActivationFunctionType
ALU = mybir.AluOpType
DR = mybir.MatmulPerfMode.DoubleRow


def _build_kernel(tc: tile.TileContext, out_ap, xT_in, ident_in):
    nc = tc.nc
    with (
        tc.tile_pool(name="sb", bufs=1) as sb,
        tc.tile_pool(name="psmm", bufs=2, space="PSUM") as psmm,
    ):
        xT = sb.tile([P, KC, N2], FP8, name="xT")
        # stream the key matrix in column pieces so matmuls can start early;
        # the first piece is small to cut the pipeline-fill latency
        splits = [0, 1024, 2048, 4096, N2]
        for lo, hi in zip(splits, splits[1:]):
            nc.sync.dma_start(out=xT[:, :, lo:hi], in_=xT_in[:, :, lo:hi])
        ident = sb.tile([P, P], BF16, name="ident")
        nc.sync.dma_start(out=ident, in_=ident_in)

        rsums = sb.tile([P, T_SLAB * NU], F32, name="rsums")
        outb = sb.tile([P, 2 * T_SLAB], F32, name="outb")

        for u in range(NU):
            for m in range(T_SLAB):
                ps = psmm.tile([P, CW], F32, tag="ps", name="ps")
                for h in range(CW // MMW):
                    off = u * CW + h * MMW
                    nc.tensor.matmul(ps[:, h * MMW:(h + 1) * MMW],
                                     xT[:, :, m * P:(m + 1) * P],
                                     xT[:, :, off:off + MMW],
                                     start=True, stop=True, perf_mode=DR)
                # psum holds 16*cos; exp(2*cos) = exp(psum * 0.125).
                # Row-sums ride the DVE STT accumulator, except the last
                # T_SLAB-1 chunks which use the ACT accumulator (the DVE
                # backlog would otherwise trail the final exp).
                ri = rsums[:, (m * NU + u):(m * NU + u) + 1]
                esc = sb.tile([P, CW], BF16, tag="esc", name="esc", bufs=12)
                if u == NU - 1 and m >= 1:
                    nc.scalar.activation(esc, ps, AF.Exp,
                                         scale=2.0 / (FP8_SCALE ** 2),
                                         accum_out=ri)
                else:
                    nc.scalar.activation(esc, ps, AF.Exp,
                                         scale=2.0 / (FP8_SCALE ** 2))
                    esc2 = sb.tile([P, CW], BF16, tag="esc2", name="esc2",
                                   bufs=4)
                    nc.vector.scalar_tensor_tensor(
                        esc2, esc, 1.0, esc, ALU.mult, ALU.max, accum_out=ri)
                if u == 2:
                    # positive diagonal (cols 4096+m*128..+128) from the exp'd
                    # chunk: host recovers 2*cos with a log
                    scr = sb.tile([P, P], BF16, tag="scr", name="scr", bufs=2)
                    nc.vector.tensor_mul(scr, esc[:, m * P:(m + 1) * P],
                                         ident)
                    nc.vector.reduce_sum(outb[:, T_SLAB + m:T_SLAB + m + 1],
                                         scr, axis=mybir.AxisListType.X)
                if u == NU - 1:
                    nc.vector.reduce_sum(outb[:, m:m + 1],
                                         rsums[:, m * NU:(m + 1) * NU],
                                         axis=mybir.AxisListType.X)
        nc.sync.dma_start(out=out_ap, in_=outb)


def build_nc():
    nc = bacc.Bacc("TRN2", target_bir_lowering=False, debug=False,
                   num_devices=NCORES)
    xT_in = nc.dram_tensor("xT", [P, KC, N2], FP8, kind="ExternalInput").ap()
    ident_in = nc.dram_tensor("ident", [P, P], BF16,
                              kind="ExternalInput").ap()
    out = nc.dram_tensor("out", [P, 2 * T_SLAB], F32,
                         kind="ExternalOutput").ap()
    with tile.TileContext(nc) as tc:
        _build_kernel(tc, out, xT_in, ident_in)
    nc.compile()
    return nc


_NC = None
LAST_RESULTS = None
_IDENT = np.eye(P, dtype=np.float32).astype(ml_dtypes.bfloat16)


def _make_in_maps(rep: np.ndarray):
    norm = np.maximum(np.sqrt((rep.astype(np.float64) ** 2).sum(1,
                                                                keepdims=True)),
                      1e-8)
    xh8 = (rep * (FP8_SCALE / norm)).astype(ml_dtypes.float8_e4m3)
    in_maps = []
    for c in range(NCORES):
        rot = np.roll(xh8, -c * ROWS, axis=0)  # col j = global row j + c*1024
        # xT[d, k, j] = rot[j, k*128 + d]
        xT = np.ascontiguousarray(
            rot.reshape(N2, KC, P).transpose(2, 1, 0))
        in_maps.append({"xT": xT, "ident": _IDENT})
    return in_maps


def kernel(representation: np.ndarray, **run_kwargs) -> np.ndarray:
    global _NC, LAST_RESULTS
    rep = np.ascontiguousarray(np.asarray(representation), dtype=np.float32)
    assert rep.shape == (N2, D)
    if _NC is None:
        _NC = build_nc()
    res = run_bass_kernel_spmd(_NC, _make_in_maps(rep),
                               core_ids=list(range(NCORES)), **run_kwargs)
    LAST_RESULTS = res
    total = 0.0
    e2 = float(np.exp(2.0))
    for r in res.results:
        out = r["out"].astype(np.float64)
        S = out[:, :T_SLAB]
        pos_exp = out[:, T_SLAB:]          # = exp(2*cos) of the positive pair
        total += float((np.log(S - e2) - np.log(pos_exp)).sum())
    return np.asarray(np.float32(total / N2))


# revision 11
# speedup vs baseline: 4.6236x; 1.0065x over previous
"""NT-Xent (SimCLR) contrastive loss on 8 Trainium2 NeuronCores.

Math: with x_hat = row-normalized representation [8192, 256], tau = 0.5,
  sim = x_hat @ x_hat.T
  loss = (1/8192) * sum_i [ ln(sum_{j!=i} exp(2 sim[i,j])) - 2 sim[i, pos(i)] ]
where pos(i) = (i + 4096) mod 8192.

Sharding: data-parallel over rows; core c owns rows [c*1024, (c+1)*1024).
The host pre-normalizes rows, quantizes to fp8e4m3 (scaled by 4), and ships
each core a pre-transposed, row-rotated key matrix xT [128, 2, 8192] so a
single SPMD program works on every core: column j of core c's slab is global
row (j + c*1024) % 8192, which puts each core's own rows at columns 0..1023
(the matmul stationary tiles) and every core's positive diagonal at columns
4096..5119.

On device, per core: 128 fp8 DoubleRow matmuls (K=256 in one pass) build the
[1024, 8192] similarity slab in [128, 2048] PSUM chunks; the ACT engine does
exp (the scale folds the 1/16 fp8 scaling and 1/tau); row-sums ride the DVE
scalar_tensor_tensor accumulator except the last few chunks, which use the
ACT accumulator so the kernel doesn't end DVE-bound. The positive diagonal is
read lazily from the exp'd u=2 chunk (identity mask + reduce; the host takes
ln to recover 2*cos). Output is [128, 16] per core; the host finishes with
ln(S - e^2) - ln(pos_exp) summed over rows.
"""

import numpy as np
import ml_dtypes

import concourse.bacc as bacc
import concourse.bass as bass
import concourse.tile as tile
from concourse import mybir
from concourse.bass_utils import run_bass_kernel_spmd

N2 = 8192            # total rows (2N)
D = 256              # feature dim
NCORES = 8
ROWS = N2 // NCORES  # 1024 rows per core
N = N2 // 2          # positive-pair offset
P = 128              # SBUF partitions
KC = 2               # two 128-row contraction chunks (K=256 via DoubleRow)
T_SLAB = ROWS // P   # 8 row tiles of the slab
CW = 2048            # exp chunk width (4 PSUM banks)
NU = N2 // CW        # 4 chunks per slab row tile
MMW = 512            # matmul moving free width (1 PSUM bank)
FP8_SCALE = 4.0      # x_hat quantized as x_hat * 4 -> sim psum = 16*cos

F32 = mybir.dt.float32
BF16 = mybir.dt.bfloat16
FP8 = mybir.dt.float8e4
AF = mybir.ActivationFunctionType
ALU = mybir.AluOpType
DR = mybir.MatmulPerfMode.DoubleRow


def _build_kernel(tc: tile.TileContext, out_ap, xT_in, ident_in):
    nc = tc.nc
    with (
        tc.tile_pool(name="sb", bufs=1) as sb,
        tc.tile_pool(name="psmm", bufs=2, space="PSUM") as psmm,
    ):
        xT = sb.tile([P, KC, N2], FP8, name="xT")
        # stream the key matrix in column pieces so matmuls can start early;
        # the first piece is small to cut the pipeline-fill latency
        splits = [0, 512, 1024, 2048, 4096, N2]
        for lo, hi in zip(splits, splits[1:]):
            nc.sync.dma_start(out=xT[:, :, lo:hi], in_=xT_in[:, :, lo:hi])
        ident = sb.tile([P, P], BF16, name="ident")
        nc.sync.dma_start(out=ident, in_=ident_in)

        rsums = sb.tile([P, T_SLAB * NU + 1], F32, name="rsums")
        outb = sb.tile([P, 2 * T_SLAB], F32, name="outb")

        # the very first chunk is emitted as two 1024-wide halves so the
        # first exp only waits on the first two DMA pieces; its two partial
        # row-sums land in rsums cols 0 and 1 (all later chunks shift by +1)
        for u in range(NU):
            for m in range(T_SLAB):
                first = (u == 0 and m == 0)
                subw = CW // 2 if first else CW
                for s0 in range(CW // subw):
                    ps = psmm.tile([P, subw], F32, tag="ps", name="ps")
                    for h in range(subw // MMW):
                        off = u * CW + s0 * subw + h * MMW
                        nc.tensor.matmul(ps[:, h * MMW:(h + 1) * MMW],
                                         xT[:, :, m * P:(m + 1) * P],
                                         xT[:, :, off:off + MMW],
                                         start=True, stop=True, perf_mode=DR)
                    # psum holds 16*cos; exp(2*cos) = exp(psum * 0.125).
                    # Row-sums ride the DVE STT accumulator, except the last
                    # T_SLAB-1 chunks which use the ACT accumulator (the DVE
                    # backlog would otherwise trail the final exp).
                    ci = s0 if first else m * NU + u + 1
                    ri = rsums[:, ci:ci + 1]
                    esc = sb.tile([P, CW], BF16, tag="esc", name="esc",
                                  bufs=12)
                    if u == NU - 1 and m >= 1:
                        nc.scalar.activation(esc[:, :subw], ps, AF.Exp,
                                             scale=2.0 / (FP8_SCALE ** 2),
                                             accum_out=ri)
                    else:
                        nc.scalar.activation(esc[:, :subw], ps, AF.Exp,
                                             scale=2.0 / (FP8_SCALE ** 2))
                        esc2 = sb.tile([P, CW], BF16, tag="esc2",
                                       name="esc2", bufs=4)
                        nc.vector.scalar_tensor_tensor(
                            esc2[:, :subw], esc[:, :subw], 1.0,
                            esc[:, :subw], ALU.mult, ALU.max, accum_out=ri)
                if u == 2:
                    # positive diagonal (cols 4096+m*128..+128) from the exp'd
                    # chunk: host recovers 2*cos with a log
                    scr = sb.tile([P, P], BF16, tag="scr", name="scr", bufs=2)
                    nc.vector.tensor_mul(scr, esc[:, m * P:(m + 1) * P],
                                         ident)
                    nc.vector.reduce_sum(outb[:, T_SLAB + m:T_SLAB + m + 1],
                                         scr, axis=mybir.AxisListType.X)
                if u == NU - 1:
                    lo = 0 if m == 0 else m * NU + 1
                    nc.vector.reduce_sum(outb[:, m:m + 1],
                                         rsums[:, lo:(m + 1) * NU + 1],
                                         axis=mybir.AxisListType.X)
        nc.sync.dma_start(out=out_ap, in_=outb)


def build_nc():
    nc = bacc.Bacc("TRN2", target_bir_lowering=False, debug=False,
                   num_devices=NCORES)
    xT_in = nc.dram_tensor("xT", [P, KC, N2], FP8, kind="ExternalInput").ap()
    ident_in = nc.dram_tensor("ident", [P, P], BF16,
                              kind="ExternalInput").ap()
    out = nc.dram_tensor("out", [P, 2 * T_SLAB], F32,
                         kind="ExternalOutput").ap()
    with tile.TileContext(nc) as tc:
        _build_kernel(tc, out, xT_in, ident_in)
    nc.compile()
    return nc


_NC = None
LAST_RESULTS = None
_IDENT = np.eye(P, dtype=np.float32).astype(ml_dtypes.bfloat16)


def _make_in_maps(rep: np.ndarray):
    norm = np.maximum(np.sqrt((rep.astype(np.float64) ** 2).sum(1,
                                                                keepdims=True)),
                      1e-8)
    xh8 = (rep * (FP8_SCALE / norm)).astype(ml_dtypes.float8_e4m3)
    in_maps = []
    for c in range(NCORES):
        rot = np.roll(xh8, -c * ROWS, axis=0)  # col j = global row j + c*1024
        # xT[d, k, j] = rot[j, k*128 + d]
        xT = np.ascontiguousarray(
            rot.reshape(N2, KC, P).transpose(2, 1, 0))
        in_maps.append({"xT": xT, "ident": _IDENT})
    return in_maps


def kernel(representation: np.ndarray, **run_kwargs) -> np.ndarray:
    global _NC, LAST_RESULTS
    rep = np.ascontiguousarray(np.asarray(representation), dtype=np.float32)
    assert rep.shape == (N2, D)
    if _NC is None:
        _NC = build_nc()
    res = run_bass_kernel_spmd(_NC, _make_in_maps(rep),
                               core_ids=list(range(NCORES)), **run_kwargs)
    LAST_RESULTS = res
    total = 0.0
    e2 = float(np.exp(2.0))
    for r in res.results:
        out = r["out"].astype(np.float64)
        S = out[:, :T_SLAB]
        pos_exp = out[:, T_SLAB:]          # = exp(2*cos) of the positive pair
        total += float((np.log(S - e2) - np.log(pos_exp)).sum())
    return np.asarray(np.float32(total / N2))
